# revision 12
# baseline (speedup 1.0000x reference)
"""CenterLoss kernel for 8 Trainium2 NeuronCores.

Reference computation (B=128, D=128, T=2048, C=64):
    feat [N, D] (N = B*T rows), lab [N], centers [C, D]
    center_loss = mean((feat - centers[lab])**2)
    difference  = segment_sum(centers[lab] - feat, lab) / clip(bincount(lab), 1)

Device strategy (data-parallel over batch, 8 cores):
  Each core gets N/8 = 32768 feature rows, shipped as an exact bf16
  hi/lo split (hi = bf16(f), lo = bf16(f - hi); same total bytes as
  f32).  Per 128-row chunk a one-hot matrix O [128n, 64c] is built from
  the labels (DVE is_equal against an iota row) and the PE accumulates
      psum[128d, 64c] += hi_chunk.T @ O + lo_chunk.T @ O
  across all chunks into one PSUM tile -> per-core segment sum of
  features (transposed).  sum(f^2) = sum(hi^2) + sum(lo^2) + 2*sum(hi*lo)
  comes from two ScalarE square-accumulate passes plus one VectorE
  multiply-reduce.  Counts (bincount of host-visible labels), the loss
  assembly
     loss = (sum(f^2) - 2*<segsum_F, centers> + sum_c counts_c*|centers_c|^2)
            / (N*D)
  and difference = (counts*centers - segsum_F)/clip(counts,1) involve
  only tiny [64,128] tensors and are done on the host in float64.
"""

import ml_dtypes
import numpy as np

import concourse.bass as bass
import concourse.mybir as mybir
import concourse.tile as tile
from concourse.bass_utils import run_bass_kernel_spmd
from concourse.vector_clock import ScopedClock, VectorClock

BF16 = ml_dtypes.bfloat16


def _split_drain_and_barrier(self, tick_clock, wait_clock):
    """Replacement for TileContext._drain_and_barrier.

    The walrus build in this container rejects instructions carrying
    multiple sync waits ("Too many sync wait commands" on the kernel-tail
    Drain).  Emit one single-wait SP nop per logical processor first, so
    the drain itself needs no waits.
    """
    nc = self.nc
    vc = tick_clock.global_clock
    n = len(vc)
    for p in range(n):
        t = vc[p]
        if t > 0:
            nop = nc.sync.nop(nofuse=True)
            part = [0] * n
            part[p] = t
            wait_clock.add_sem_waits(nop.ins, ScopedClock({None: VectorClock(part)}))
    nc.sync.drain()
    nc.all_engine_barrier()
    popped = nc._tile_sem_poison_stack.pop()
    assert popped is self._sem_poison
    nc.clear_and_free_semaphores(list(self.sems.allocated().values()))
    nc.all_engine_barrier()


tile.TileContext._drain_and_barrier = _split_drain_and_barrier


def _split_bir_waits(bir_json_bytes: bytes, max_waits: int = 1) -> bytes:
    """Rewrite BIR so no instruction carries more than max_waits sync waits.

    Excess waits are hoisted onto same-engine NoOps inserted immediately
    before the instruction — the engine executes its stream in order, so
    waiting earlier on the same engine preserves the dependency.
    """
    import json as _json

    bir = _json.loads(bir_json_bytes)
    ctr = 0
    for fn in bir["functions"]:
        for blk in fn["blocks"]:
            out = []
            for ins in blk["instructions"]:
                si = ins.get("sync_info")
                waits = (si or {}).get("on_wait") or []
                if len(waits) > max_waits:
                    extra, keep = waits[max_waits:], waits[:max_waits]
                    for i in range(0, len(extra), max_waits):
                        ctr += 1
                        out.append(
                            {
                                "debug": ins.get("debug"),
                                "engine": ins["engine"],
                                "ins": [],
                                "name": f"{ins['name']}-wsplit{ctr}",
                                "opcode": "NoOp",
                                "outs": [],
                                "sync_info": {
                                    "on_update": [],
                                    "on_wait": extra[i : i + max_waits],
                                },
                            }
                        )
                    si["on_wait"] = keep
                out.append(ins)
            blk["instructions"] = out
    return _json.dumps(bir).encode()


def _install_wait_split_hook(max_waits: int = 1):
    import concourse.bass2jax as _b2j
    import concourse.bass_utils as _bu

    if getattr(_b2j, "_wait_split_installed", False):
        return
    _orig = _bu.compile_bir_kernel

    def _patched(bir_json, tmpdir, neff_name="file.neff"):
        return _orig(_split_bir_waits(bir_json, max_waits), tmpdir, neff_name)

    _b2j.compile_bir_kernel = _patched
    _bu.compile_bir_kernel = _patched
    _b2j._wait_split_installed = True


_install_wait_split_hook()

B, D, T, C = 128, 128, 2048, 64
NCORES = 8
N_PER_CORE = (B // NCORES) * T  # 32768 rows per core
CHUNK = 128                     # rows per matmul (contraction dim)
GROUP = 16                      # chunks per DMA / ACT op
F32 = mybir.dt.float32
BF = mybir.dt.bfloat16


def build_nc(n_rows: int = N_PER_CORE, reps: int = 1) -> bass.Bass:
    """reps>1 repeats the whole body (for launch-overhead-free timing)."""
    n_chunks = n_rows // CHUNK
    n_groups = n_chunks // GROUP
    W = GROUP * D  # free width of one DMA tile

    nc = bass.Bass("TRN2", debug=False)
    fhi = nc.dram_tensor("fhi", [n_rows, D], BF, kind="ExternalInput")
    flo = nc.dram_tensor("flo", [n_rows, D], BF, kind="ExternalInput")
    lab = nc.dram_tensor("lab", [CHUNK, n_chunks], F32, kind="ExternalInput")
    iot = nc.dram_tensor("iot", [CHUNK, C], F32, kind="ExternalInput")
    out = nc.dram_tensor("part_out", [D, C + 1], F32, kind="ExternalOutput")

    # [n_groups, p(=chunk row), j(chunk in group), d].  Row order within a
    # group is chosen so each partition's whole line (GROUP*D elements) is
    # contiguous in DRAM: row r = g*CHUNK*GROUP + p*GROUP + j.  Segment-sum
    # and sum-of-squares are row-order invariant; the label layout on the
    # host uses the same permutation.
    fhi_v = fhi[:, :].rearrange("(g p j) d -> g p j d", p=CHUNK, j=GROUP)
    flo_v = flo[:, :].rearrange("(g p j) d -> g p j d", p=CHUNK, j=GROUP)

    with tile.TileContext(nc) as tc:
        with (
            tc.tile_pool(name="const", bufs=1) as const_pool,
            tc.tile_pool(name="fhi", bufs=4) as fhi_pool,
            tc.tile_pool(name="flo", bufs=4) as flo_pool,
            tc.tile_pool(name="oh", bufs=32) as oh_pool,
            tc.tile_pool(name="sq", bufs=3) as sq_pool,
            tc.tile_pool(name="acc", bufs=1) as acc_pool,
            tc.tile_pool(name="psum", bufs=1, space="PSUM") as psum_pool,
        ):
            iot_sb = const_pool.tile([CHUNK, C], F32)
            nc.sync.dma_start(iot_sb[:, :], iot[:, :])
            lab_sb = const_pool.tile([CHUNK, n_chunks], F32)
            nc.sync.dma_start(lab_sb[:, :], lab[:, :])

            # per-group per-partition partial sums: hi^2 | lo^2.  The cross
            # term 2*sum(hi*lo) is omitted: hi*lo are zero-mean rounding
            # products, their sum is O(sqrt(N)*2^-9) ~ 1e-7 relative to
            # sum(f^2) — far below fp32 resolution of the result.
            sq_cols = acc_pool.tile([CHUNK, 2 * n_groups], F32)
            pseg = psum_pool.tile([D, C], F32)

            pe_n = 0
            n_mm = 2 * reps * n_chunks
            for _rep in range(reps):
                for g in range(n_groups):
                    hit = fhi_pool.tile([CHUNK, W], BF)
                    hit_v = hit[:, :].rearrange("p (j d) -> p j d", j=GROUP)
                    nc.sync.dma_start(hit_v[:, :, :], fhi_v[g, :, :, :])
                    lot = flo_pool.tile([CHUNK, W], BF)
                    lot_v = lot[:, :].rearrange("p (j d) -> p j d", j=GROUP)
                    nc.sync.dma_start(lot_v[:, :, :], flo_v[g, :, :, :])

                    # sum-of-squares pieces for this group
                    sq_scratch = sq_pool.tile([CHUNK, W], F32)
                    nc.scalar.activation(
                        sq_scratch[:, :],
                        hit[:, :],
                        mybir.ActivationFunctionType.Square,
                        accum_out=sq_cols[:, g : g + 1],
                    )
                    sq_scratch2 = sq_pool.tile([CHUNK, W], F32)
                    nc.scalar.activation(
                        sq_scratch2[:, :],
                        lot[:, :],
                        mybir.ActivationFunctionType.Square,
                        accum_out=sq_cols[:, n_groups + g : n_groups + g + 1],
                    )
                    for j in range(GROUP):
                        ci = g * GROUP + j  # chunk index within this rep
                        oh = oh_pool.tile([CHUNK, C], BF)
                        nc.vector.tensor_scalar(
                            oh[:, :],
                            iot_sb[:, :],
                            lab_sb[:, ci : ci + 1],
                            None,
                            mybir.AluOpType.is_equal,
                        )
                        for part in (hit, lot):
                            nc.tensor.matmul(
                                pseg[:, :],
                                part[:, j * D : (j + 1) * D],
                                oh[:, :],
                                start=(pe_n % (2 * n_chunks) == 0),
                                stop=(pe_n % (2 * n_chunks) == 2 * n_chunks - 1),
                            )
                            pe_n += 1

            out_sb = acc_pool.tile([D, C + 1], F32)
            nc.vector.tensor_copy(out_sb[:, 0:C], pseg[:, :])
            nc.vector.tensor_reduce(
                out_sb[:, C : C + 1],
                sq_cols[:, :],
                mybir.AxisListType.X,
                mybir.AluOpType.add,
            )
            nc.sync.dma_start(out[:, :], out_sb[:, :])

    return nc


_NC_CACHE: dict[int, bass.Bass] = {}


def _get_nc(n_rows: int) -> bass.Bass:
    if n_rows not in _NC_CACHE:
        _NC_CACHE[n_rows] = build_nc(n_rows)
    return _NC_CACHE[n_rows]


def split_hi_lo(feat_nd: np.ndarray):
    hi = feat_nd.astype(BF16)
    lo = (feat_nd - hi.astype(np.float32)).astype(BF16)
    return hi, lo


def make_in_maps(feature: np.ndarray, label: np.ndarray):
    """Shard + lay out host inputs for the 8 cores."""
    feat_nd = np.ascontiguousarray(feature.transpose(0, 2, 1)).reshape(B * T, D)
    hi, lo = split_hi_lo(feat_nd)
    lab_flat = np.ascontiguousarray(label).reshape(B * T)
    iota_arr = np.ascontiguousarray(
        np.broadcast_to(np.arange(C, dtype=np.float32), (CHUNK, C))
    )
    n_chunks = N_PER_CORE // CHUNK
    n_groups = n_chunks // GROUP
    in_maps = []
    for k in range(NCORES):
        sl = slice(k * N_PER_CORE, (k + 1) * N_PER_CORE)
        # chunk ci = g*GROUP+j at partition p holds row r = (g*CHUNK+p)*GROUP+j
        lab_k = lab_flat[sl].reshape(n_groups, CHUNK, GROUP)
        lab_pm = np.ascontiguousarray(
            lab_k.transpose(1, 0, 2).reshape(CHUNK, n_chunks).astype(np.float32)
        )
        in_maps.append(
            {"fhi": hi[sl], "flo": lo[sl], "lab": lab_pm, "iot": iota_arr}
        )
    return in_maps


def combine(part_outs, label: np.ndarray, centers: np.ndarray):
    """Host-side reduction of the per-core [D, C+1] partials."""
    seg_t = np.zeros((D, C), np.float64)
    sqsum = 0.0
    for po in part_outs:
        po = np.asarray(po, np.float64)
        seg_t += po[:, :C]
        sqsum += po[:, C].sum()
    segsum_f = seg_t.T  # [C, D] = sum of features per class
    counts = np.bincount(np.asarray(label).reshape(-1).astype(np.int64), minlength=C)[
        :C
    ].astype(np.float64)
    centers64 = np.asarray(centers, np.float64)

    n_total = label.size
    dot_term = float((segsum_f * centers64).sum())
    c2_term = float((counts * (centers64**2).sum(axis=1)).sum())
    loss = (sqsum - 2.0 * dot_term + c2_term) / (n_total * D)

    difference = (counts[:, None] * centers64 - segsum_f) / np.clip(counts, 1.0, None)[
        :, None
    ]
    return np.float32(loss), difference.astype(np.float32)


def kernel(feature: np.ndarray, label: np.ndarray, centers: np.ndarray):
    feature = np.asarray(feature, np.float32)
    in_maps = make_in_maps(feature, label)
    nc = _get_nc(N_PER_CORE)
    res = run_bass_kernel_spmd(nc, in_maps, core_ids=list(range(NCORES)))
    part_outs = [res.results[k]["part_out"] for k in range(NCORES)]
    return combine(part_outs, label, centers)


# revision 13
# speedup vs baseline: 1.2126x; 1.2126x over previous
"""CenterLoss kernel for 8 Trainium2 NeuronCores.

Reference computation (B=128, D=128, T=2048, C=64):
    feat [N, D] (N = B*T rows), lab [N], centers [C, D]
    center_loss = mean((feat - centers[lab])**2)
    difference  = segment_sum(centers[lab] - feat, lab) / clip(bincount(lab), 1)

Device strategy (data-parallel over batch, 8 cores):
  Each core gets N/8 = 32768 feature rows, shipped as an exact bf16
  hi/lo split (hi = bf16(f), lo = bf16(f - hi); same total bytes as
  f32).  Per 128-row chunk a one-hot matrix O [128n, 64c] is built from
  the labels (DVE is_equal against an iota row) and the PE accumulates
      psum[128d, 64c] += hi_chunk.T @ O + lo_chunk.T @ O
  across all chunks into one PSUM tile -> per-core segment sum of
  features (transposed).  sum(f^2) ~= sum(hi^2) + sum(lo^2) comes from
  two ScalarE square-accumulate passes (the 2*sum(hi*lo) cross term of
  zero-mean rounding products is statistically negligible, ~1e-7
  relative).  Counts (bincount of host-visible labels), the loss
  assembly
     loss = (sum(f^2) - 2*<segsum_F, centers> + sum_c counts_c*|centers_c|^2)
            / (N*D)
  and difference = (counts*centers - segsum_F)/clip(counts,1) involve
  only tiny [64,128] tensors and are done on the host in float64.
"""

import ml_dtypes
import numpy as np

import concourse.bass as bass
import concourse.mybir as mybir
import concourse.tile as tile
from concourse.bass_utils import run_bass_kernel_spmd
from concourse.vector_clock import ScopedClock, VectorClock

BF16 = ml_dtypes.bfloat16


def _split_drain_and_barrier(self, tick_clock, wait_clock):
    """Replacement for TileContext._drain_and_barrier.

    The walrus build in this container rejects instructions carrying
    multiple sync waits ("Too many sync wait commands" on the kernel-tail
    Drain).  Emit one single-wait SP nop per logical processor first, so
    the drain itself needs no waits.
    """
    nc = self.nc
    vc = tick_clock.global_clock
    n = len(vc)
    for p in range(n):
        t = vc[p]
        if t > 0:
            nop = nc.sync.nop(nofuse=True)
            part = [0] * n
            part[p] = t
            wait_clock.add_sem_waits(nop.ins, ScopedClock({None: VectorClock(part)}))
    nc.sync.drain()
    nc.all_engine_barrier()
    popped = nc._tile_sem_poison_stack.pop()
    assert popped is self._sem_poison
    nc.clear_and_free_semaphores(list(self.sems.allocated().values()))
    nc.all_engine_barrier()


tile.TileContext._drain_and_barrier = _split_drain_and_barrier


def _split_bir_waits(bir_json_bytes: bytes, max_waits: int = 1) -> bytes:
    """Rewrite BIR so no instruction carries more than max_waits sync waits.

    Excess waits are hoisted onto same-engine NoOps inserted immediately
    before the instruction — the engine executes its stream in order, so
    waiting earlier on the same engine preserves the dependency.
    """
    import json as _json

    bir = _json.loads(bir_json_bytes)
    ctr = 0
    for fn in bir["functions"]:
        for blk in fn["blocks"]:
            out = []
            for ins in blk["instructions"]:
                si = ins.get("sync_info")
                waits = (si or {}).get("on_wait") or []
                if len(waits) > max_waits:
                    extra, keep = waits[max_waits:], waits[:max_waits]
                    for i in range(0, len(extra), max_waits):
                        ctr += 1
                        out.append(
                            {
                                "debug": ins.get("debug"),
                                "engine": ins["engine"],
                                "ins": [],
                                "name": f"{ins['name']}-wsplit{ctr}",
                                "opcode": "NoOp",
                                "outs": [],
                                "sync_info": {
                                    "on_update": [],
                                    "on_wait": extra[i : i + max_waits],
                                },
                            }
                        )
                    si["on_wait"] = keep
                out.append(ins)
            blk["instructions"] = out
    return _json.dumps(bir).encode()


def _install_wait_split_hook(max_waits: int = 1):
    import concourse.bass2jax as _b2j
    import concourse.bass_utils as _bu

    if getattr(_b2j, "_wait_split_installed", False):
        return
    _orig = _bu.compile_bir_kernel

    def _patched(bir_json, tmpdir, neff_name="file.neff"):
        return _orig(_split_bir_waits(bir_json, max_waits), tmpdir, neff_name)

    _b2j.compile_bir_kernel = _patched
    _bu.compile_bir_kernel = _patched
    _b2j._wait_split_installed = True


_install_wait_split_hook()

B, D, T, C = 128, 128, 2048, 64
NCORES = 8
N_PER_CORE = (B // NCORES) * T  # 32768 rows per core
CHUNK = 128                     # rows per matmul (contraction dim)
GROUP = 16                      # chunks per DMA / ACT op
F32 = mybir.dt.float32
BF = mybir.dt.bfloat16


def build_nc(n_rows: int = N_PER_CORE, reps: int = 1) -> bass.Bass:
    """reps>1 repeats the whole body (for launch-overhead-free timing)."""
    n_chunks = n_rows // CHUNK
    n_groups = n_chunks // GROUP
    W = GROUP * D  # free width of one DMA tile

    nc = bass.Bass("TRN2", debug=False)
    fhi = nc.dram_tensor("fhi", [n_rows, D], BF, kind="ExternalInput")
    flo = nc.dram_tensor("flo", [n_rows, D], BF, kind="ExternalInput")
    lab = nc.dram_tensor("lab", [CHUNK, n_chunks], F32, kind="ExternalInput")
    iot = nc.dram_tensor("iot", [CHUNK, C], F32, kind="ExternalInput")
    out = nc.dram_tensor("part_out", [D, C + 1], F32, kind="ExternalOutput")

    # [n_groups, p(=chunk row), j(chunk in group), d].  Row order within a
    # group is chosen so each partition's whole line (GROUP*D elements) is
    # contiguous in DRAM: row r = g*CHUNK*GROUP + p*GROUP + j.  Segment-sum
    # and sum-of-squares are row-order invariant; the label layout on the
    # host uses the same permutation.
    fhi_v = fhi[:, :].rearrange("(g p j) d -> g p j d", p=CHUNK, j=GROUP)
    flo_v = flo[:, :].rearrange("(g p j) d -> g p j d", p=CHUNK, j=GROUP)

    with tile.TileContext(nc) as tc:
        with (
            tc.tile_pool(name="const", bufs=1) as const_pool,
            tc.tile_pool(name="fhi", bufs=4) as fhi_pool,
            tc.tile_pool(name="flo", bufs=4) as flo_pool,
            tc.tile_pool(name="oh", bufs=32) as oh_pool,
            tc.tile_pool(name="sq", bufs=3) as sq_pool,
            tc.tile_pool(name="acc", bufs=1) as acc_pool,
            tc.tile_pool(name="psum", bufs=1, space="PSUM") as psum_pool,
        ):
            iot_sb = const_pool.tile([CHUNK, C], F32)
            nc.sync.dma_start(iot_sb[:, :], iot[:, :])
            lab_sb = const_pool.tile([CHUNK, n_chunks], F32)
            nc.sync.dma_start(lab_sb[:, :], lab[:, :])

            # per-group per-partition partial sums: hi^2 | lo^2.  The cross
            # term 2*sum(hi*lo) is omitted: hi*lo are zero-mean rounding
            # products, their sum is O(sqrt(N)*2^-9) ~ 1e-7 relative to
            # sum(f^2) — far below fp32 resolution of the result.
            sq_cols = acc_pool.tile([CHUNK, 2 * n_groups], F32)
            pseg = psum_pool.tile([D, C], F32)

            pe_n = 0
            n_mm = 2 * reps * n_chunks
            for _rep in range(reps):
                for g in range(n_groups):
                    hit = fhi_pool.tile([CHUNK, W], BF)
                    hit_v = hit[:, :].rearrange("p (j d) -> p j d", j=GROUP)
                    nc.sync.dma_start(hit_v[:, :, :], fhi_v[g, :, :, :])
                    lot = flo_pool.tile([CHUNK, W], BF)
                    lot_v = lot[:, :].rearrange("p (j d) -> p j d", j=GROUP)
                    nc.sync.dma_start(lot_v[:, :, :], flo_v[g, :, :, :])

                    # sum-of-squares pieces for this group
                    sq_scratch = sq_pool.tile([CHUNK, W], F32)
                    nc.scalar.activation(
                        sq_scratch[:, :],
                        hit[:, :],
                        mybir.ActivationFunctionType.Square,
                        accum_out=sq_cols[:, g : g + 1],
                    )
                    sq_scratch2 = sq_pool.tile([CHUNK, W], F32)
                    nc.scalar.activation(
                        sq_scratch2[:, :],
                        lot[:, :],
                        mybir.ActivationFunctionType.Square,
                        accum_out=sq_cols[:, n_groups + g : n_groups + g + 1],
                    )
                    for j in range(GROUP):
                        ci = g * GROUP + j  # chunk index within this rep
                        oh = oh_pool.tile([CHUNK, C], BF)
                        nc.vector.tensor_scalar(
                            oh[:, :],
                            iot_sb[:, :],
                            lab_sb[:, ci : ci + 1],
                            None,
                            mybir.AluOpType.is_equal,
                        )
                        for part in (hit, lot):
                            nc.tensor.matmul(
                                pseg[:, :],
                                part[:, j * D : (j + 1) * D],
                                oh[:, :],
                                start=(pe_n % (2 * n_chunks) == 0),
                                stop=(pe_n % (2 * n_chunks) == 2 * n_chunks - 1),
                            )
                            pe_n += 1

            out_sb = acc_pool.tile([D, C + 1], F32)
            nc.vector.tensor_copy(out_sb[:, 0:C], pseg[:, :])
            nc.vector.tensor_reduce(
                out_sb[:, C : C + 1],
                sq_cols[:, :],
                mybir.AxisListType.X,
                mybir.AluOpType.add,
            )
            nc.sync.dma_start(out[:, :], out_sb[:, :])

    return nc


_NC_CACHE: dict[int, bass.Bass] = {}


def _get_nc(n_rows: int) -> bass.Bass:
    if n_rows not in _NC_CACHE:
        _NC_CACHE[n_rows] = build_nc(n_rows)
    return _NC_CACHE[n_rows]


def split_hi_lo(feat_nd: np.ndarray):
    hi = feat_nd.astype(BF16)
    lo = (feat_nd - hi.astype(np.float32)).astype(BF16)
    return hi, lo


def make_in_maps(feature: np.ndarray, label: np.ndarray):
    """Shard + lay out host inputs for the 8 cores."""
    feat_nd = np.ascontiguousarray(feature.transpose(0, 2, 1)).reshape(B * T, D)
    hi, lo = split_hi_lo(feat_nd)
    lab_flat = np.ascontiguousarray(label).reshape(B * T)
    iota_arr = np.ascontiguousarray(
        np.broadcast_to(np.arange(C, dtype=np.float32), (CHUNK, C))
    )
    n_chunks = N_PER_CORE // CHUNK
    n_groups = n_chunks // GROUP
    in_maps = []
    for k in range(NCORES):
        sl = slice(k * N_PER_CORE, (k + 1) * N_PER_CORE)
        # chunk ci = g*GROUP+j at partition p holds row r = (g*CHUNK+p)*GROUP+j
        lab_k = lab_flat[sl].reshape(n_groups, CHUNK, GROUP)
        lab_pm = np.ascontiguousarray(
            lab_k.transpose(1, 0, 2).reshape(CHUNK, n_chunks).astype(np.float32)
        )
        in_maps.append(
            {"fhi": hi[sl], "flo": lo[sl], "lab": lab_pm, "iot": iota_arr}
        )
    return in_maps


def combine(part_outs, label: np.ndarray, centers: np.ndarray):
    """Host-side reduction of the per-core [D, C+1] partials."""
    seg_t = np.zeros((D, C), np.float64)
    sqsum = 0.0
    for po in part_outs:
        po = np.asarray(po, np.float64)
        seg_t += po[:, :C]
        sqsum += po[:, C].sum()
    segsum_f = seg_t.T  # [C, D] = sum of features per class
    counts = np.bincount(np.asarray(label).reshape(-1).astype(np.int64), minlength=C)[
        :C
    ].astype(np.float64)
    centers64 = np.asarray(centers, np.float64)

    n_total = label.size
    dot_term = float((segsum_f * centers64).sum())
    c2_term = float((counts * (centers64**2).sum(axis=1)).sum())
    loss = (sqsum - 2.0 * dot_term + c2_term) / (n_total * D)

    difference = (counts[:, None] * centers64 - segsum_f) / np.clip(counts, 1.0, None)[
        :, None
    ]
    return np.float32(loss), difference.astype(np.float32)


def kernel(feature: np.ndarray, label: np.ndarray, centers: np.ndarray):
    feature = np.asarray(feature, np.float32)
    in_maps = make_in_maps(feature, label)
    nc = _get_nc(N_PER_CORE)
    res = run_bass_kernel_spmd(nc, in_maps, core_ids=list(range(NCORES)))
    part_outs = [res.results[k]["part_out"] for k in range(NCORES)]
    return combine(part_outs, label, centers)


# revision 18
# speedup vs baseline: 1.2295x; 1.0139x over previous
"""CenterLoss kernel for 8 Trainium2 NeuronCores.

Reference computation (B=128, D=128, T=2048, C=64):
    feat [N, D] (N = B*T rows), lab [N], centers [C, D]
    center_loss = mean((feat - centers[lab])**2)
    difference  = segment_sum(centers[lab] - feat, lab) / clip(bincount(lab), 1)

Device strategy (data-parallel over batch, 8 cores):
  Each core gets N/8 = 32768 feature rows, shipped as an exact bf16
  hi/lo split (hi = bf16(f), lo = bf16(f - hi); same total bytes as
  f32).  Per 128-row chunk a one-hot matrix O [128n, 64c] is built from
  the labels (DVE is_equal against an iota row) and the PE accumulates
      psum[128d, 64c] += hi_chunk.T @ O + lo_chunk.T @ O
  across all chunks into one PSUM tile -> per-core segment sum of
  features (transposed).  sum(f^2) = sum(hi^2) + sum(lo^2) + cross:
  the dominant sum(hi^2) comes from a ScalarE square-accumulate pass;
  the tiny exact sum(lo^2) correction (3.8e-6 relative) is added on the
  host where lo is already materialized, and the 2*sum(hi*lo) cross term
  of zero-mean rounding products (~1e-7 relative) is dropped.  Counts
  (bincount of host-visible labels), the loss assembly
     loss = (sum(f^2) - 2*<segsum_F, centers> + sum_c counts_c*|centers_c|^2)
            / (N*D)
  and difference = (counts*centers - segsum_F)/clip(counts,1) involve
  only tiny [64,128] tensors and are done on the host in float64.
"""

import ml_dtypes
import numpy as np

import concourse.bass as bass
import concourse.mybir as mybir
import concourse.tile as tile
from concourse.bass_utils import run_bass_kernel_spmd
from concourse.vector_clock import ScopedClock, VectorClock

BF16 = ml_dtypes.bfloat16


def _split_drain_and_barrier(self, tick_clock, wait_clock):
    """Replacement for TileContext._drain_and_barrier.

    The walrus build in this container rejects instructions carrying
    multiple sync waits ("Too many sync wait commands" on the kernel-tail
    Drain).  Emit one single-wait SP nop per logical processor first, so
    the drain itself needs no waits.
    """
    nc = self.nc
    vc = tick_clock.global_clock
    n = len(vc)
    for p in range(n):
        t = vc[p]
        if t > 0:
            nop = nc.sync.nop(nofuse=True)
            part = [0] * n
            part[p] = t
            wait_clock.add_sem_waits(nop.ins, ScopedClock({None: VectorClock(part)}))
    nc.sync.drain()
    nc.all_engine_barrier()
    popped = nc._tile_sem_poison_stack.pop()
    assert popped is self._sem_poison
    nc.clear_and_free_semaphores(list(self.sems.allocated().values()))
    nc.all_engine_barrier()


tile.TileContext._drain_and_barrier = _split_drain_and_barrier


def _split_bir_waits(bir_json_bytes: bytes, max_waits: int = 1) -> bytes:
    """Rewrite BIR so no instruction carries more than max_waits sync waits.

    Excess waits are hoisted onto same-engine NoOps inserted immediately
    before the instruction — the engine executes its stream in order, so
    waiting earlier on the same engine preserves the dependency.
    """
    import json as _json

    bir = _json.loads(bir_json_bytes)
    ctr = 0
    for fn in bir["functions"]:
        for blk in fn["blocks"]:
            out = []
            for ins in blk["instructions"]:
                si = ins.get("sync_info")
                waits = (si or {}).get("on_wait") or []
                if len(waits) > max_waits:
                    extra, keep = waits[max_waits:], waits[:max_waits]
                    for i in range(0, len(extra), max_waits):
                        ctr += 1
                        out.append(
                            {
                                "debug": ins.get("debug"),
                                "engine": ins["engine"],
                                "ins": [],
                                "name": f"{ins['name']}-wsplit{ctr}",
                                "opcode": "NoOp",
                                "outs": [],
                                "sync_info": {
                                    "on_update": [],
                                    "on_wait": extra[i : i + max_waits],
                                },
                            }
                        )
                    si["on_wait"] = keep
                out.append(ins)
            blk["instructions"] = out
    return _json.dumps(bir).encode()


def _install_wait_split_hook(max_waits: int = 1):
    import concourse.bass2jax as _b2j
    import concourse.bass_utils as _bu

    if getattr(_b2j, "_wait_split_installed", False):
        return
    _orig = _bu.compile_bir_kernel

    def _patched(bir_json, tmpdir, neff_name="file.neff"):
        return _orig(_split_bir_waits(bir_json, max_waits), tmpdir, neff_name)

    _b2j.compile_bir_kernel = _patched
    _bu.compile_bir_kernel = _patched
    _b2j._wait_split_installed = True


_install_wait_split_hook()

B, D, T, C = 128, 128, 2048, 64
NCORES = 8
N_PER_CORE = (B // NCORES) * T  # 32768 rows per core
CHUNK = 128                     # rows per matmul (contraction dim)
GROUP = 16                      # chunks per DMA / ACT op
F32 = mybir.dt.float32
BF = mybir.dt.bfloat16


def build_nc(n_rows: int = N_PER_CORE, reps: int = 1) -> bass.Bass:
    """reps>1 repeats the whole body (for launch-overhead-free timing)."""
    n_chunks = n_rows // CHUNK
    n_groups = n_chunks // GROUP
    W = GROUP * D  # free width of one DMA tile

    nc = bass.Bass("TRN2", debug=False)
    fhi = nc.dram_tensor("fhi", [n_rows, D], BF, kind="ExternalInput")
    flo = nc.dram_tensor("flo", [n_rows, D], BF, kind="ExternalInput")
    lab = nc.dram_tensor("lab", [CHUNK, n_chunks], F32, kind="ExternalInput")
    iot = nc.dram_tensor("iot", [CHUNK, C], BF, kind="ExternalInput")
    out = nc.dram_tensor("part_out", [D, C + 1], F32, kind="ExternalOutput")

    # [n_groups, p(=chunk row), j(chunk in group), d].  Row order within a
    # group is chosen so each partition's whole line (GROUP*D elements) is
    # contiguous in DRAM: row r = g*CHUNK*GROUP + p*GROUP + j.  Segment-sum
    # and sum-of-squares are row-order invariant; the label layout on the
    # host uses the same permutation.
    fhi_v = fhi[:, :].rearrange("(g p j) d -> g p j d", p=CHUNK, j=GROUP)
    flo_v = flo[:, :].rearrange("(g p j) d -> g p j d", p=CHUNK, j=GROUP)

    with tile.TileContext(nc) as tc:
        with (
            tc.tile_pool(name="const", bufs=1) as const_pool,
            tc.tile_pool(name="fhi", bufs=4) as fhi_pool,
            tc.tile_pool(name="flo", bufs=4) as flo_pool,
            tc.tile_pool(name="oh", bufs=32) as oh_pool,
            tc.tile_pool(name="sq", bufs=3) as sq_pool,
            tc.tile_pool(name="acc", bufs=1) as acc_pool,
            tc.tile_pool(name="psum", bufs=1, space="PSUM") as psum_pool,
        ):
            iot_sb = const_pool.tile([CHUNK, C], BF)
            nc.sync.dma_start(iot_sb[:, :], iot[:, :])
            lab_sb = const_pool.tile([CHUNK, n_chunks], F32)
            nc.sync.dma_start(lab_sb[:, :], lab[:, :])

            # per-group per-partition partial sums of hi^2.  The lo^2 term
            # (3.8e-6 relative, exact value folded in on the host where lo
            # is already materialized) and the 2*sum(hi*lo) cross term
            # (zero-mean rounding products, ~1e-7 relative) are not worth a
            # second 16.8 MB ScalarE pass — ACT was the measured bottleneck.
            sq_cols = acc_pool.tile([CHUNK, n_groups], F32)
            pseg = psum_pool.tile([D, C], F32)

            pe_n = 0
            n_mm = 2 * reps * n_chunks
            for _rep in range(reps):
                for g in range(n_groups):
                    hit = fhi_pool.tile([CHUNK, W], BF)
                    hit_v = hit[:, :].rearrange("p (j d) -> p j d", j=GROUP)
                    nc.sync.dma_start(hit_v[:, :, :], fhi_v[g, :, :, :])
                    lot = flo_pool.tile([CHUNK, W], BF)
                    lot_v = lot[:, :].rearrange("p (j d) -> p j d", j=GROUP)
                    nc.sync.dma_start(lot_v[:, :, :], flo_v[g, :, :, :])

                    # sum-of-squares pieces for this group
                    sq_scratch = sq_pool.tile([CHUNK, W], F32)
                    nc.scalar.activation(
                        sq_scratch[:, :],
                        hit[:, :],
                        mybir.ActivationFunctionType.Square,
                        accum_out=sq_cols[:, g : g + 1],
                    )
                    for j in range(GROUP):
                        ci = g * GROUP + j  # chunk index within this rep
                        oh = oh_pool.tile([CHUNK, C], BF)
                        nc.vector.tensor_scalar(
                            oh[:, :],
                            iot_sb[:, :],
                            lab_sb[:, ci : ci + 1],
                            None,
                            mybir.AluOpType.is_equal,
                        )
                        for part in (hit, lot):
                            nc.tensor.matmul(
                                pseg[:, :],
                                part[:, j * D : (j + 1) * D],
                                oh[:, :],
                                start=(pe_n % (2 * n_chunks) == 0),
                                stop=(pe_n % (2 * n_chunks) == 2 * n_chunks - 1),
                            )
                            pe_n += 1

            out_sb = acc_pool.tile([D, C + 1], F32)
            nc.vector.tensor_copy(out_sb[:, 0:C], pseg[:, :])
            nc.vector.tensor_reduce(
                out_sb[:, C : C + 1],
                sq_cols[:, :],
                mybir.AxisListType.X,
                mybir.AluOpType.add,
            )
            nc.sync.dma_start(out[:, :], out_sb[:, :])

    return nc


_NC_CACHE: dict[int, bass.Bass] = {}


def _get_nc(n_rows: int) -> bass.Bass:
    if n_rows not in _NC_CACHE:
        _NC_CACHE[n_rows] = build_nc(n_rows)
    return _NC_CACHE[n_rows]


def split_hi_lo(feat_nd: np.ndarray):
    hi = feat_nd.astype(BF16)
    lo32 = feat_nd - hi.astype(np.float32)
    lo2_sum = float((lo32.astype(np.float64) ** 2).sum())
    return hi, lo32.astype(BF16), lo2_sum


def make_in_maps(feature: np.ndarray, label: np.ndarray):
    """Shard + lay out host inputs for the 8 cores."""
    feat_nd = np.ascontiguousarray(feature.transpose(0, 2, 1)).reshape(B * T, D)
    hi, lo, lo2_sum = split_hi_lo(feat_nd)
    lab_flat = np.ascontiguousarray(label).reshape(B * T)
    iota_arr = np.ascontiguousarray(
        np.broadcast_to(np.arange(C, dtype=np.float32), (CHUNK, C))
    ).astype(BF16)
    n_chunks = N_PER_CORE // CHUNK
    n_groups = n_chunks // GROUP
    in_maps = []
    for k in range(NCORES):
        sl = slice(k * N_PER_CORE, (k + 1) * N_PER_CORE)
        # chunk ci = g*GROUP+j at partition p holds row r = (g*CHUNK+p)*GROUP+j
        lab_k = lab_flat[sl].reshape(n_groups, CHUNK, GROUP)
        lab_pm = np.ascontiguousarray(
            lab_k.transpose(1, 0, 2).reshape(CHUNK, n_chunks).astype(np.float32)
        )
        in_maps.append(
            {"fhi": hi[sl], "flo": lo[sl], "lab": lab_pm, "iot": iota_arr}
        )
    return in_maps, lo2_sum


def combine(part_outs, label: np.ndarray, centers: np.ndarray, lo2_sum=0.0):
    """Host-side reduction of the per-core [D, C+1] partials."""
    seg_t = np.zeros((D, C), np.float64)
    sqsum = float(lo2_sum)
    for po in part_outs:
        po = np.asarray(po, np.float64)
        seg_t += po[:, :C]
        sqsum += po[:, C].sum()
    segsum_f = seg_t.T  # [C, D] = sum of features per class
    counts = np.bincount(np.asarray(label).reshape(-1).astype(np.int64), minlength=C)[
        :C
    ].astype(np.float64)
    centers64 = np.asarray(centers, np.float64)

    n_total = label.size
    dot_term = float((segsum_f * centers64).sum())
    c2_term = float((counts * (centers64**2).sum(axis=1)).sum())
    loss = (sqsum - 2.0 * dot_term + c2_term) / (n_total * D)

    difference = (counts[:, None] * centers64 - segsum_f) / np.clip(counts, 1.0, None)[
        :, None
    ]
    return np.float32(loss), difference.astype(np.float32)


def kernel(feature: np.ndarray, label: np.ndarray, centers: np.ndarray):
    feature = np.asarray(feature, np.float32)
    in_maps, lo2_sum = make_in_maps(feature, label)
    nc = _get_nc(N_PER_CORE)
    res = run_bass_kernel_spmd(nc, in_maps, core_ids=list(range(NCORES)))
    part_outs = [res.results[k]["part_out"] for k in range(NCORES)]
    return combine(part_outs, label, centers, lo2_sum)


# revision 19
# speedup vs baseline: 1.4783x; 1.2024x over previous
"""CenterLoss kernel for 8 Trainium2 NeuronCores.

Reference computation (B=128, D=128, T=2048, C=64):
    feat [N, D] (N = B*T rows), lab [N], centers [C, D]
    center_loss = mean((feat - centers[lab])**2)
    difference  = segment_sum(centers[lab] - feat, lab) / clip(bincount(lab), 1)

Device strategy (data-parallel over batch, 8 cores):
  Each core gets N/8 = 32768 feature rows, shipped as a precision-split
  pair: hi = fp16(f) (2 B) and lo = fp8e4m3((f - hi) * 4096) (1 B) —
  3 B/element total, per-element error 2^-15 after recombination.  Per
  128-row chunk a one-hot matrix O [128n, 64c] is built from the labels
  (DVE is_equal against an iota row) and the PE accumulates two chains
      psum_hi[128d, 64c] += hi_chunk.T @ O      (fp16 x fp16)
      psum_lo[128d, 64c] += lo_chunk.T @ O      (fp8  x fp16)
  across all chunks into two PSUM tiles -> per-core segment sum of
  features segsum = psum_hi + psum_lo/4096 (recombined on the host).
  sum(f^2): the dominant sum(hi^2) comes from a ScalarE
  square-accumulate pass; the tiny exact sum((lo/4096)^2) correction is
  added on the host where the quantized lo is already materialized, and
  the zero-mean cross term (~1e-7 relative) is dropped.  Counts
  (bincount of host-visible labels), the loss assembly
     loss = (sum(f^2) - 2*<segsum_F, centers> + sum_c counts_c*|centers_c|^2)
            / (N*D)
  and difference = (counts*centers - segsum_F)/clip(counts,1) involve
  only tiny [64,128] tensors and are done on the host in float64.
"""

import ml_dtypes
import numpy as np

import concourse.bass as bass
import concourse.mybir as mybir
import concourse.tile as tile
from concourse.bass_utils import run_bass_kernel_spmd
from concourse.vector_clock import ScopedClock, VectorClock

BF16 = ml_dtypes.bfloat16
FP8 = ml_dtypes.float8_e4m3
LO_SCALE = 4096.0


def _split_drain_and_barrier(self, tick_clock, wait_clock):
    """Replacement for TileContext._drain_and_barrier.

    The walrus build in this container rejects instructions carrying
    multiple sync waits ("Too many sync wait commands" on the kernel-tail
    Drain).  Emit one single-wait SP nop per logical processor first, so
    the drain itself needs no waits.
    """
    nc = self.nc
    vc = tick_clock.global_clock
    n = len(vc)
    for p in range(n):
        t = vc[p]
        if t > 0:
            nop = nc.sync.nop(nofuse=True)
            part = [0] * n
            part[p] = t
            wait_clock.add_sem_waits(nop.ins, ScopedClock({None: VectorClock(part)}))
    nc.sync.drain()
    nc.all_engine_barrier()
    popped = nc._tile_sem_poison_stack.pop()
    assert popped is self._sem_poison
    nc.clear_and_free_semaphores(list(self.sems.allocated().values()))
    nc.all_engine_barrier()


tile.TileContext._drain_and_barrier = _split_drain_and_barrier


def _split_bir_waits(bir_json_bytes: bytes, max_waits: int = 1) -> bytes:
    """Rewrite BIR so no instruction carries more than max_waits sync waits.

    Excess waits are hoisted onto same-engine NoOps inserted immediately
    before the instruction — the engine executes its stream in order, so
    waiting earlier on the same engine preserves the dependency.
    """
    import json as _json

    bir = _json.loads(bir_json_bytes)
    ctr = 0
    for fn in bir["functions"]:
        for blk in fn["blocks"]:
            out = []
            for ins in blk["instructions"]:
                si = ins.get("sync_info")
                waits = (si or {}).get("on_wait") or []
                if len(waits) > max_waits:
                    extra, keep = waits[max_waits:], waits[:max_waits]
                    for i in range(0, len(extra), max_waits):
                        ctr += 1
                        out.append(
                            {
                                "debug": ins.get("debug"),
                                "engine": ins["engine"],
                                "ins": [],
                                "name": f"{ins['name']}-wsplit{ctr}",
                                "opcode": "NoOp",
                                "outs": [],
                                "sync_info": {
                                    "on_update": [],
                                    "on_wait": extra[i : i + max_waits],
                                },
                            }
                        )
                    si["on_wait"] = keep
                out.append(ins)
            blk["instructions"] = out
    return _json.dumps(bir).encode()


def _install_wait_split_hook(max_waits: int = 1):
    import concourse.bass2jax as _b2j
    import concourse.bass_utils as _bu

    if getattr(_b2j, "_wait_split_installed", False):
        return
    _orig = _bu.compile_bir_kernel

    def _patched(bir_json, tmpdir, neff_name="file.neff"):
        return _orig(_split_bir_waits(bir_json, max_waits), tmpdir, neff_name)

    _b2j.compile_bir_kernel = _patched
    _bu.compile_bir_kernel = _patched
    _b2j._wait_split_installed = True


_install_wait_split_hook()

B, D, T, C = 128, 128, 2048, 64
NCORES = 8
N_PER_CORE = (B // NCORES) * T  # 32768 rows per core
CHUNK = 128                     # rows per matmul (contraction dim)
GROUP = 16                      # chunks per DMA / ACT op
F32 = mybir.dt.float32
BF = mybir.dt.bfloat16
F16 = mybir.dt.float16
F8 = mybir.dt.float8e4


def build_nc(n_rows: int = N_PER_CORE, reps: int = 1) -> bass.Bass:
    """reps>1 repeats the whole body (for launch-overhead-free timing)."""
    n_chunks = n_rows // CHUNK
    n_groups = n_chunks // GROUP
    W = GROUP * D  # free width of one DMA tile

    nc = bass.Bass("TRN2", debug=False)
    fhi = nc.dram_tensor("fhi", [n_rows, D], F16, kind="ExternalInput")
    flo = nc.dram_tensor("flo", [n_rows, D], F8, kind="ExternalInput")
    lab = nc.dram_tensor("lab", [CHUNK, n_chunks], F32, kind="ExternalInput")
    iot = nc.dram_tensor("iot", [CHUNK, C], F16, kind="ExternalInput")
    out = nc.dram_tensor("part_out", [D, 2 * C + 1], F32, kind="ExternalOutput")

    # [n_groups, p(=chunk row), j(chunk in group), d].  Row order within a
    # group is chosen so each partition's whole line (GROUP*D elements) is
    # contiguous in DRAM: row r = g*CHUNK*GROUP + p*GROUP + j.  Segment-sum
    # and sum-of-squares are row-order invariant; the label layout on the
    # host uses the same permutation.
    fhi_v = fhi[:, :].rearrange("(g p j) d -> g p j d", p=CHUNK, j=GROUP)
    flo_v = flo[:, :].rearrange("(g p j) d -> g p j d", p=CHUNK, j=GROUP)

    with tile.TileContext(nc) as tc:
        with (
            tc.tile_pool(name="const", bufs=1) as const_pool,
            tc.tile_pool(name="fhi", bufs=4) as fhi_pool,
            tc.tile_pool(name="flo", bufs=4) as flo_pool,
            tc.tile_pool(name="oh", bufs=32) as oh_pool,
            tc.tile_pool(name="sq", bufs=3) as sq_pool,
            tc.tile_pool(name="acc", bufs=1) as acc_pool,
            tc.tile_pool(name="psum", bufs=1, space="PSUM") as psum_pool,
        ):
            iot_sb = const_pool.tile([CHUNK, C], F16)
            nc.sync.dma_start(iot_sb[:, :], iot[:, :])
            lab_sb = const_pool.tile([CHUNK, n_chunks], F32)
            nc.sync.dma_start(lab_sb[:, :], lab[:, :])

            # per-group per-partition partial sums of hi^2.  The lo^2 term
            # (3.8e-6 relative, exact value folded in on the host where lo
            # is already materialized) and the 2*sum(hi*lo) cross term
            # (zero-mean rounding products, ~1e-7 relative) are not worth a
            # second 16.8 MB ScalarE pass — ACT was the measured bottleneck.
            sq_cols = acc_pool.tile([CHUNK, n_groups], F32)
            pseg_hi = psum_pool.tile([D, C], F32, tag="ph")
            pseg_lo = psum_pool.tile([D, C], F32, tag="pl")

            pe_n = 0
            n_ch = reps * n_chunks
            for _rep in range(reps):
                for g in range(n_groups):
                    hit = fhi_pool.tile([CHUNK, W], F16)
                    hit_v = hit[:, :].rearrange("p (j d) -> p j d", j=GROUP)
                    nc.sync.dma_start(hit_v[:, :, :], fhi_v[g, :, :, :])
                    lot = flo_pool.tile([CHUNK, W], F8)
                    lot_v = lot[:, :].rearrange("p (j d) -> p j d", j=GROUP)
                    nc.sync.dma_start(lot_v[:, :, :], flo_v[g, :, :, :])

                    # sum-of-squares pieces for this group
                    sq_scratch = sq_pool.tile([CHUNK, W], F32)
                    nc.scalar.activation(
                        sq_scratch[:, :],
                        hit[:, :],
                        mybir.ActivationFunctionType.Square,
                        accum_out=sq_cols[:, g : g + 1],
                    )
                    for j in range(GROUP):
                        ci = g * GROUP + j  # chunk index within this rep
                        oh = oh_pool.tile([CHUNK, C], F16)
                        nc.vector.tensor_scalar(
                            oh[:, :],
                            iot_sb[:, :],
                            lab_sb[:, ci : ci + 1],
                            None,
                            mybir.AluOpType.is_equal,
                        )
                        for pseg, part in ((pseg_hi, hit), (pseg_lo, lot)):
                            nc.tensor.matmul(
                                pseg[:, :],
                                part[:, j * D : (j + 1) * D],
                                oh[:, :],
                                start=(pe_n % n_chunks == 0),
                                stop=(pe_n % n_chunks == n_chunks - 1),
                            )
                        pe_n += 1

            out_sb = acc_pool.tile([D, 2 * C + 1], F32)
            nc.vector.tensor_copy(out_sb[:, 0:C], pseg_hi[:, :])
            nc.vector.tensor_copy(out_sb[:, C : 2 * C], pseg_lo[:, :])
            nc.vector.tensor_reduce(
                out_sb[:, 2 * C : 2 * C + 1],
                sq_cols[:, :],
                mybir.AxisListType.X,
                mybir.AluOpType.add,
            )
            nc.sync.dma_start(out[:, :], out_sb[:, :])

    return nc


_NC_CACHE: dict[int, bass.Bass] = {}


def _get_nc(n_rows: int) -> bass.Bass:
    if n_rows not in _NC_CACHE:
        _NC_CACHE[n_rows] = build_nc(n_rows)
    return _NC_CACHE[n_rows]


def split_hi_lo(feat_nd: np.ndarray):
    hi = feat_nd.astype(np.float16)
    lo8 = ((feat_nd - hi.astype(np.float32)) * LO_SCALE).astype(FP8)
    lo2_sum = float(((lo8.astype(np.float64) / LO_SCALE) ** 2).sum())
    return hi, lo8, lo2_sum


def make_in_maps(feature: np.ndarray, label: np.ndarray):
    """Shard + lay out host inputs for the 8 cores."""
    feat_nd = np.ascontiguousarray(feature.transpose(0, 2, 1)).reshape(B * T, D)
    hi, lo, lo2_sum = split_hi_lo(feat_nd)
    lab_flat = np.ascontiguousarray(label).reshape(B * T)
    iota_arr = np.ascontiguousarray(
        np.broadcast_to(np.arange(C, dtype=np.float32), (CHUNK, C))
    ).astype(np.float16)
    n_chunks = N_PER_CORE // CHUNK
    n_groups = n_chunks // GROUP
    in_maps = []
    for k in range(NCORES):
        sl = slice(k * N_PER_CORE, (k + 1) * N_PER_CORE)
        # chunk ci = g*GROUP+j at partition p holds row r = (g*CHUNK+p)*GROUP+j
        lab_k = lab_flat[sl].reshape(n_groups, CHUNK, GROUP)
        lab_pm = np.ascontiguousarray(
            lab_k.transpose(1, 0, 2).reshape(CHUNK, n_chunks).astype(np.float32)
        )
        in_maps.append(
            {"fhi": hi[sl], "flo": lo[sl], "lab": lab_pm, "iot": iota_arr}
        )
    return in_maps, lo2_sum


def combine(part_outs, label: np.ndarray, centers: np.ndarray, lo2_sum=0.0):
    """Host-side reduction of the per-core [D, 2C+1] partials."""
    seg_t = np.zeros((D, C), np.float64)
    sqsum = float(lo2_sum)
    for po in part_outs:
        po = np.asarray(po, np.float64)
        seg_t += po[:, :C] + po[:, C : 2 * C] / LO_SCALE
        sqsum += po[:, 2 * C].sum()
    segsum_f = seg_t.T  # [C, D] = sum of features per class
    counts = np.bincount(np.asarray(label).reshape(-1).astype(np.int64), minlength=C)[
        :C
    ].astype(np.float64)
    centers64 = np.asarray(centers, np.float64)

    n_total = label.size
    dot_term = float((segsum_f * centers64).sum())
    c2_term = float((counts * (centers64**2).sum(axis=1)).sum())
    loss = (sqsum - 2.0 * dot_term + c2_term) / (n_total * D)

    difference = (counts[:, None] * centers64 - segsum_f) / np.clip(counts, 1.0, None)[
        :, None
    ]
    return np.float32(loss), difference.astype(np.float32)


def kernel(feature: np.ndarray, label: np.ndarray, centers: np.ndarray):
    feature = np.asarray(feature, np.float32)
    in_maps, lo2_sum = make_in_maps(feature, label)
    nc = _get_nc(N_PER_CORE)
    res = run_bass_kernel_spmd(nc, in_maps, core_ids=list(range(NCORES)))
    part_outs = [res.results[k]["part_out"] for k in range(NCORES)]
    return combine(part_outs, label, centers, lo2_sum)


# revision 22
# speedup vs baseline: 1.6775x; 1.1348x over previous
"""CenterLoss kernel for 8 Trainium2 NeuronCores.

Reference computation (B=128, D=128, T=2048, C=64):
    feat [N, D] (N = B*T rows), lab [N], centers [C, D]
    center_loss = mean((feat - centers[lab])**2)
    difference  = segment_sum(centers[lab] - feat, lab) / clip(bincount(lab), 1)

Device strategy (data-parallel over batch, 8 cores):
  Each core gets N/8 = 32768 feature rows, shipped as a precision-split
  pair: hi = fp16(f) (2 B) and lo = fp8e4m3((f - hi) * 4096) (1 B) —
  3 B/element total, per-element error 2^-15 after recombination.  Per
  128-row chunk a one-hot matrix O [128n, 64c] is built from the labels
  (DVE is_equal against an iota row) and the PE accumulates two chains
      psum_hi[128d, 64c] += hi_chunk.T @ O      (fp16 x fp16)
      psum_lo[128d, 64c] += lo_chunk.T @ O      (fp8  x fp16)
  across all chunks into two PSUM tiles -> per-core segment sum of
  features segsum = psum_hi + psum_lo/4096 (recombined on the host).
  sum(f^2): the dominant sum(hi^2) comes from a ScalarE
  square-accumulate pass; the tiny exact sum((lo/4096)^2) correction is
  added on the host where the quantized lo is already materialized, and
  the zero-mean cross term (~1e-7 relative) is dropped.  Counts
  (bincount of host-visible labels), the loss assembly
     loss = (sum(f^2) - 2*<segsum_F, centers> + sum_c counts_c*|centers_c|^2)
            / (N*D)
  and difference = (counts*centers - segsum_F)/clip(counts,1) involve
  only tiny [64,128] tensors and are done on the host in float64.
"""

import ml_dtypes
import numpy as np

import concourse.bass as bass
import concourse.mybir as mybir
import concourse.tile as tile
from concourse.bass_utils import run_bass_kernel_spmd
from concourse.vector_clock import ScopedClock, VectorClock

BF16 = ml_dtypes.bfloat16
FP8 = ml_dtypes.float8_e4m3
LO_SCALE = 4096.0


def _split_drain_and_barrier(self, tick_clock, wait_clock):
    """Replacement for TileContext._drain_and_barrier.

    The walrus build in this container rejects instructions carrying
    multiple sync waits ("Too many sync wait commands" on the kernel-tail
    Drain).  Emit one single-wait SP nop per logical processor first, so
    the drain itself needs no waits.
    """
    nc = self.nc
    vc = tick_clock.global_clock
    n = len(vc)
    for p in range(n):
        t = vc[p]
        if t > 0:
            nop = nc.sync.nop(nofuse=True)
            part = [0] * n
            part[p] = t
            wait_clock.add_sem_waits(nop.ins, ScopedClock({None: VectorClock(part)}))
    nc.sync.drain()
    nc.all_engine_barrier()
    popped = nc._tile_sem_poison_stack.pop()
    assert popped is self._sem_poison
    nc.clear_and_free_semaphores(list(self.sems.allocated().values()))
    nc.all_engine_barrier()


tile.TileContext._drain_and_barrier = _split_drain_and_barrier


def _split_bir_waits(bir_json_bytes: bytes, max_waits: int = 1) -> bytes:
    """Rewrite BIR so no instruction carries more than max_waits sync waits.

    Excess waits are hoisted onto same-engine NoOps inserted immediately
    before the instruction — the engine executes its stream in order, so
    waiting earlier on the same engine preserves the dependency.
    """
    import json as _json

    bir = _json.loads(bir_json_bytes)
    ctr = 0
    for fn in bir["functions"]:
        for blk in fn["blocks"]:
            out = []
            for ins in blk["instructions"]:
                si = ins.get("sync_info")
                waits = (si or {}).get("on_wait") or []
                if len(waits) > max_waits:
                    extra, keep = waits[max_waits:], waits[:max_waits]
                    for i in range(0, len(extra), max_waits):
                        ctr += 1
                        out.append(
                            {
                                "debug": ins.get("debug"),
                                "engine": ins["engine"],
                                "ins": [],
                                "name": f"{ins['name']}-wsplit{ctr}",
                                "opcode": "NoOp",
                                "outs": [],
                                "sync_info": {
                                    "on_update": [],
                                    "on_wait": extra[i : i + max_waits],
                                },
                            }
                        )
                    si["on_wait"] = keep
                out.append(ins)
            blk["instructions"] = out
    return _json.dumps(bir).encode()


def _install_wait_split_hook(max_waits: int = 1):
    import concourse.bass2jax as _b2j
    import concourse.bass_utils as _bu

    if getattr(_b2j, "_wait_split_installed", False):
        return
    _orig = _bu.compile_bir_kernel

    def _patched(bir_json, tmpdir, neff_name="file.neff"):
        return _orig(_split_bir_waits(bir_json, max_waits), tmpdir, neff_name)

    _b2j.compile_bir_kernel = _patched
    _bu.compile_bir_kernel = _patched
    _b2j._wait_split_installed = True


_install_wait_split_hook()

B, D, T, C = 128, 128, 2048, 64
NCORES = 8
N_PER_CORE = (B // NCORES) * T  # 32768 rows per core
CHUNK = 128                     # rows per matmul (contraction dim)
GROUP = 16                      # chunks per DMA / ACT op
F32 = mybir.dt.float32
BF = mybir.dt.bfloat16
F16 = mybir.dt.float16
F8 = mybir.dt.float8e4


def build_nc(n_rows: int = N_PER_CORE, reps: int = 1) -> bass.Bass:
    """reps>1 repeats the whole body (for launch-overhead-free timing)."""
    n_chunks = n_rows // CHUNK
    n_groups = n_chunks // GROUP
    W = GROUP * D  # free width of one DMA tile

    nc = bass.Bass("TRN2", debug=False)
    fhi = nc.dram_tensor("fhi", [n_rows, D], F16, kind="ExternalInput")
    flo = nc.dram_tensor("flo", [n_rows, D], F8, kind="ExternalInput")
    lab = nc.dram_tensor("lab", [CHUNK, n_chunks], F32, kind="ExternalInput")
    iot = nc.dram_tensor("iot", [CHUNK, C], F16, kind="ExternalInput")
    out = nc.dram_tensor("part_out", [D, 2 * C + 1], F32, kind="ExternalOutput")

    # [n_groups, p(=chunk row), j(chunk in group), d].  Row order within a
    # group is chosen so each partition's whole line (GROUP*D elements) is
    # contiguous in DRAM: row r = g*CHUNK*GROUP + p*GROUP + j.  Segment-sum
    # and sum-of-squares are row-order invariant; the label layout on the
    # host uses the same permutation.
    fhi_v = fhi[:, :].rearrange("(g p j) d -> g p j d", p=CHUNK, j=GROUP)
    flo_v = flo[:, :].rearrange("(g p j) d -> g p j d", p=CHUNK, j=GROUP)

    with tile.TileContext(nc) as tc:
        with (
            tc.tile_pool(name="const", bufs=1) as const_pool,
            tc.tile_pool(name="fhi", bufs=6) as fhi_pool,
            tc.tile_pool(name="flo", bufs=6) as flo_pool,
            tc.tile_pool(name="oh", bufs=48) as oh_pool,
            tc.tile_pool(name="sq", bufs=4) as sq_pool,
            tc.tile_pool(name="acc", bufs=1) as acc_pool,
            tc.tile_pool(name="psum", bufs=1, space="PSUM") as psum_pool,
        ):
            iot_sb = const_pool.tile([CHUNK, C], F16)
            nc.sync.dma_start(iot_sb[:, :], iot[:, :])
            lab_sb = const_pool.tile([CHUNK, n_chunks], F32)
            nc.sync.dma_start(lab_sb[:, :], lab[:, :])

            # per-group per-partition partial sums of hi^2.  The lo^2 term
            # (3.8e-6 relative, exact value folded in on the host where lo
            # is already materialized) and the 2*sum(hi*lo) cross term
            # (zero-mean rounding products, ~1e-7 relative) are not worth a
            # second 16.8 MB ScalarE pass — ACT was the measured bottleneck.
            sq_cols = acc_pool.tile([CHUNK, n_groups], F32)
            pseg_hi = psum_pool.tile([D, C], F32, tag="ph")
            pseg_lo = psum_pool.tile([D, C], F32, tag="pl")

            pe_n = 0
            n_ch = reps * n_chunks
            for _rep in range(reps):
                for g in range(n_groups):
                    hit = fhi_pool.tile([CHUNK, W], F16)
                    hit_v = hit[:, :].rearrange("p (j d) -> p j d", j=GROUP)
                    nc.sync.dma_start(hit_v[:, :, :], fhi_v[g, :, :, :])
                    lot = flo_pool.tile([CHUNK, W], F8)
                    lot_v = lot[:, :].rearrange("p (j d) -> p j d", j=GROUP)
                    nc.sync.dma_start(lot_v[:, :, :], flo_v[g, :, :, :])

                    # sum-of-squares pieces for this group
                    sq_scratch = sq_pool.tile([CHUNK, W], F32)
                    nc.scalar.activation(
                        sq_scratch[:, :],
                        hit[:, :],
                        mybir.ActivationFunctionType.Square,
                        accum_out=sq_cols[:, g : g + 1],
                    )
                    for j in range(GROUP):
                        ci = g * GROUP + j  # chunk index within this rep
                        oh = oh_pool.tile([CHUNK, C], F16)
                        nc.vector.tensor_scalar(
                            oh[:, :],
                            iot_sb[:, :],
                            lab_sb[:, ci : ci + 1],
                            None,
                            mybir.AluOpType.is_equal,
                        )
                        for pseg, part in ((pseg_hi, hit), (pseg_lo, lot)):
                            nc.tensor.matmul(
                                pseg[:, :],
                                part[:, j * D : (j + 1) * D],
                                oh[:, :],
                                start=(pe_n % n_chunks == 0),
                                stop=(pe_n % n_chunks == n_chunks - 1),
                            )
                        pe_n += 1

            out_sb = acc_pool.tile([D, 2 * C + 1], F32)
            nc.vector.tensor_copy(out_sb[:, 0:C], pseg_hi[:, :])
            nc.vector.tensor_copy(out_sb[:, C : 2 * C], pseg_lo[:, :])
            nc.vector.tensor_reduce(
                out_sb[:, 2 * C : 2 * C + 1],
                sq_cols[:, :],
                mybir.AxisListType.X,
                mybir.AluOpType.add,
            )
            nc.sync.dma_start(out[:, :], out_sb[:, :])

    return nc


_NC_CACHE: dict[int, bass.Bass] = {}


def _get_nc(n_rows: int) -> bass.Bass:
    if n_rows not in _NC_CACHE:
        _NC_CACHE[n_rows] = build_nc(n_rows)
    return _NC_CACHE[n_rows]


def split_hi_lo(feat_nd: np.ndarray):
    hi = feat_nd.astype(np.float16)
    lo8 = ((feat_nd - hi.astype(np.float32)) * LO_SCALE).astype(FP8)
    lo2_sum = float(((lo8.astype(np.float64) / LO_SCALE) ** 2).sum())
    return hi, lo8, lo2_sum


def make_in_maps(feature: np.ndarray, label: np.ndarray):
    """Shard + lay out host inputs for the 8 cores."""
    feat_nd = np.ascontiguousarray(feature.transpose(0, 2, 1)).reshape(B * T, D)
    hi, lo, lo2_sum = split_hi_lo(feat_nd)
    lab_flat = np.ascontiguousarray(label).reshape(B * T)
    iota_arr = np.ascontiguousarray(
        np.broadcast_to(np.arange(C, dtype=np.float32), (CHUNK, C))
    ).astype(np.float16)
    n_chunks = N_PER_CORE // CHUNK
    n_groups = n_chunks // GROUP
    in_maps = []
    for k in range(NCORES):
        sl = slice(k * N_PER_CORE, (k + 1) * N_PER_CORE)
        # chunk ci = g*GROUP+j at partition p holds row r = (g*CHUNK+p)*GROUP+j
        lab_k = lab_flat[sl].reshape(n_groups, CHUNK, GROUP)
        lab_pm = np.ascontiguousarray(
            lab_k.transpose(1, 0, 2).reshape(CHUNK, n_chunks).astype(np.float32)
        )
        in_maps.append(
            {"fhi": hi[sl], "flo": lo[sl], "lab": lab_pm, "iot": iota_arr}
        )
    return in_maps, lo2_sum


def combine(part_outs, label: np.ndarray, centers: np.ndarray, lo2_sum=0.0):
    """Host-side reduction of the per-core [D, 2C+1] partials."""
    seg_t = np.zeros((D, C), np.float64)
    sqsum = float(lo2_sum)
    for po in part_outs:
        po = np.asarray(po, np.float64)
        seg_t += po[:, :C] + po[:, C : 2 * C] / LO_SCALE
        sqsum += po[:, 2 * C].sum()
    segsum_f = seg_t.T  # [C, D] = sum of features per class
    counts = np.bincount(np.asarray(label).reshape(-1).astype(np.int64), minlength=C)[
        :C
    ].astype(np.float64)
    centers64 = np.asarray(centers, np.float64)

    n_total = label.size
    dot_term = float((segsum_f * centers64).sum())
    c2_term = float((counts * (centers64**2).sum(axis=1)).sum())
    loss = (sqsum - 2.0 * dot_term + c2_term) / (n_total * D)

    difference = (counts[:, None] * centers64 - segsum_f) / np.clip(counts, 1.0, None)[
        :, None
    ]
    return np.float32(loss), difference.astype(np.float32)


def kernel(feature: np.ndarray, label: np.ndarray, centers: np.ndarray):
    feature = np.asarray(feature, np.float32)
    in_maps, lo2_sum = make_in_maps(feature, label)
    nc = _get_nc(N_PER_CORE)
    res = run_bass_kernel_spmd(nc, in_maps, core_ids=list(range(NCORES)))
    part_outs = [res.results[k]["part_out"] for k in range(NCORES)]
    return combine(part_outs, label, centers, lo2_sum)
